# revision 24
# baseline (speedup 1.0000x reference)
"""MoE routed dense layer (nn_MultiHeadDense): y[b] = x[b] @ W[idx[b]] + bias[idx[b]].

Full shapes: inputs [4096,1024] f32, indices [4096] int, kernel [8,1024,1024] f32,
bias [8,1024] f32 -> out [4096,1024] f32.

Sharding strategy (expert-parallel, H == n_cores == 8): core h owns expert h's
weight [1024,1024] and processes up to C=512 of the rows routed to expert h.
The host computes the per-expert row lists from `indices`, gathers each
expert's first C rows into a zero-padded transposed activation block
XT_h [D, C], and scatters the per-core outputs back into the full [B, F]
result. Rows beyond C on an overloaded expert (~1% of rows for balanced
routing) are computed on the host in f32; this keeps the device at exactly
4 full 128-row m-tiles (64 matmuls) instead of 5 mostly-empty ones.

On-device per core: Y[c, f] = sum_k XT[k*128:(k+1)*128, c].T @ W[k*128:.., f]
accumulated in PSUM over the 8 k-tiles, bias added during the PSUM->SBUF
eviction (cast to fp16 for the output DMA; host upcasts). X and W are
pre-cast to fp16 on the host (11-bit mantissa keeps the absmax error ~1e-3
of output scale while halving HBM traffic); accumulation stays fp32 in PSUM.

Schedule (from trace analysis of the previous versions):
- Input stream = 8 uniform sync-queue DMAs, one [P, F+C] chunk per k-tile
  (W[k] | XT[k]) in consumption order, followed by the bias (needed ~2us
  after the last chunk, so it can't delay the stream head).  Uniform small
  chunks deliver the k6/k7 data ~2us earlier than coarser groupings,
  removing the mid-stream PE stall; per-chunk completion sems pipeline the
  ~1-2us DMA completion-receipt latency.
- Zero-matmul warmup bridges PE activity from queue start until chunk0
  lands so the HAM full-duty clock (2.4 GHz after ~3.4us of sustained PE
  activity) is reached with as few cold real matmuls as possible.
- Phase 1 runs k0..k3 k-outermost (k0 n-outer so only chunk0a gates the
  first matmuls, racing the DMA fill); phase 2 runs k4..k7 m-outermost,
  n-outer within each m, with per-(m, n) evictions into single-bank PSUM
  tiles: DVE adds bias and casts PSUM fp32 -> fp16, then the [128,512]
  piece is DMA'd out immediately, n=0 on the scalar ring and n=1 on the
  sync ring.  Finishing each m's n=0 bank first lets its DVE eviction
  overlap the whole n=1 sub-window, so only the final ~0.73us eviction
  trails the last matmul; small fp16 output pieces shorten the
  last-matmul -> last-DMA-receipt tail that gates the fixed ~8.3us
  framework epilogue.
"""

from contextlib import ExitStack

import numpy as np

import concourse.bass as bass
import concourse.tile as tile
from concourse import bacc, mybir
from concourse.bass_utils import run_bass_kernel_spmd

F32 = mybir.dt.float32
F16 = mybir.dt.float16

P = 128          # SBUF partitions / matmul tile edge
NTILE = 512      # matmul moving free dim (one fp32 PSUM bank)
CAP = 512        # device rows per core; overflow rows computed on host
WARMUP_MM = 7    # zero-matmuls bridging PE idle until chunk0 lands
PHASE1_K = 4     # k0..PHASE1_K-1 run k-outer; the rest run m-outer + evict


def _build(nc: bass.Bass, C: int, D: int, F: int, warmup=WARMUP_MM):
    KT = D // P
    NT = F // NTILE
    MT = C // P
    assert C % P == 0 and NT == 2
    Q = F + C        # columns per full k-tile chunk

    wx = nc.dram_tensor("wx", (KT * P * Q,), F16, kind="ExternalInput").ap()
    bias_d = nc.dram_tensor("bias", (P * F,), F16, kind="ExternalInput").ap()
    y = nc.dram_tensor("y", (C, F), F16, kind="ExternalOutput").ap()

    with tile.TileContext(nc) as tc, ExitStack() as ctx:
        cp = ctx.enter_context(tc.tile_pool(name="cp", bufs=1))
        zp = ctx.enter_context(tc.tile_pool(name="zp", bufs=1))
        pp = ctx.enter_context(tc.tile_pool(name="pp", bufs=1, space="PSUM"))
        yp = ctx.enter_context(tc.tile_pool(name="yp", bufs=1))

        # Input stream on the sync HWDGE ring, in consumption order:
        # chunk0a [P, 512+C] = W[k0][:, :512] | XT[k0]  (k0's n=0 matmuls
        # run for all four m-tiles before anything needs chunk0b, riding
        # out the per-chunk completion-sem lag), chunk0b [P, 512] =
        # W[k0][:, 512:], then one [P, F+C] chunk (W[k] | XT[k]) per k-tile
        # k1..k7.  The bias rides the same ring AFTER the stream (it is
        # first needed ~2us after the last chunk lands) so it cannot delay
        # the head of the stream, and the scalar ring stays free for the
        # n=0 output DMAs.
        sizes = [NTILE + C, NTILE] + [Q] * (KT - 1)
        wx_c = []
        off = 0
        for c, q in enumerate(sizes):
            ct = cp.tile([P, q], F16, name=f"wx{c}", tag=f"wx{c}")
            nc.sync.dma_start(
                ct[:], wx[off:off + P * q].rearrange("(p q) -> p q", p=P))
            wx_c.append(ct)
            off += P * q
        bias_t = cp.tile([P, F], F16, name="bias", tag="bias")
        nc.sync.dma_start(
            bias_t[:], bias_d[:].rearrange("(p q) -> p q", p=P))

        # One single-bank PSUM tile per (m, n) output block: keeps the DVE
        # eviction of bank (m, n=0) free of any (false) dependency against
        # the still-running matmuls into bank (m, n=1).
        ps = [[pp.tile([P, NTILE], F32, name=f"ps{m}_{n}", tag=f"ps{m}_{n}")
               for n in range(NT)] for m in range(MT)]

        # PE warmup: zero matmuls (no DMA dependency) keep the PE busy
        # until chunk0a's completion receipt lands, so the HAM clock-gate
        # warmup overlaps the DMA fill instead of following it. They
        # target ps[0][0], which the first real k=0 matmul resets (start=True).
        zt = zp.tile([P, NTILE], F16)
        nc.vector.memset(zt[:], 0.0)
        for _ in range(warmup):
            nc.tensor.matmul(ps[0][0][:], lhsT=zt[:, :P], rhs=zt[:],
                             start=True, stop=True)

        def mm(m, k, n):
            if k == 0:
                t = wx_c[0] if n == 0 else wx_c[1]
                xt = wx_c[0]
                xbase, wbase = NTILE, 0
            else:
                t = xt = wx_c[k + 1]
                xbase, wbase = F, n * NTILE
            nc.tensor.matmul(
                ps[m][n][:],
                lhsT=xt[:, xbase + m * P:xbase + (m + 1) * P],
                rhs=t[:, wbase:wbase + NTILE],
                start=(k == 0),
                stop=(k == KT - 1),
            )

        def evict(m, n):
            yt = yp.tile([P, NTILE], F16, name=f"yt{m}_{n}", tag=f"yt{m}_{n}")
            nc.vector.tensor_add(
                yt[:],
                ps[m][n][:],
                bias_t[:, n * NTILE:(n + 1) * NTILE],
            )
            eng = nc.scalar if n == 0 else nc.sync
            eng.dma_start(
                y[m * P:(m + 1) * P, n * NTILE:(n + 1) * NTILE], yt[:])

        # k0 runs n-outer so only chunk0a gates the first four matmuls;
        # later k-tiles run n-inner (both n in one chunk).
        for n in range(NT):
            for m in range(MT):
                mm(m, 0, n)
        for k in range(1, PHASE1_K):
            for m in range(MT):
                for n in range(NT):
                    mm(m, k, n)
        # Phase 2, n-outer within each m: finish the n=0 bank first so its
        # DVE eviction overlaps the whole n=1 sub-window, leaving only the
        # n=1 eviction (~0.8us) trailing the last matmul.
        for m in range(MT):
            for n in range(NT):
                for k in range(PHASE1_K, KT):
                    mm(m, k, n)
                evict(m, n)


LAST_PROFILE = {}


def kernel(inputs, indices, kernel, bias, _trace=False):
    x = np.ascontiguousarray(np.asarray(inputs), dtype=np.float32)
    idx = np.asarray(indices).astype(np.int64)
    wk = np.asarray(kernel, dtype=np.float32)
    bv = np.asarray(bias, dtype=np.float32)

    B, D = x.shape
    H, _, F = wk.shape
    C = CAP
    KT = D // P

    rows = [np.nonzero(idx == h)[0] for h in range(H)]
    kept = [r[:C] for r in rows]
    over = [r[C:] for r in rows]

    def pack(w16, xt16):
        # stream layout: chunk0a [P, 512+C] = W[k0][:, :512] | XT[k0]
        #                chunk0b [P, 512]   = W[k0][:, 512:]
        #                chunk k [P, F+C]   = W[k] | XT[k]     (k=1..KT-1)
        w = w16.reshape(KT, P, F)
        xt = xt16.reshape(KT, P, C)
        parts = [
            np.concatenate([w[0, :, :NTILE], xt[0]], axis=1).reshape(-1),
            w[0, :, NTILE:].reshape(-1),
            np.concatenate([w[1:], xt[1:]], axis=2).reshape(-1),
        ]
        return np.concatenate(parts)

    in_maps = []
    for h in range(H):
        r = kept[h]
        xt = np.zeros((D, C), dtype=np.float16)
        xt[:, :len(r)] = x[r].T
        in_maps.append({
            "wx": pack(wk[h].astype(np.float16), xt),
            "bias": np.broadcast_to(bv[h].astype(np.float16), (P, F)).reshape(-1),
        })

    nc = bacc.Bacc(
        "TRN2", target_bir_lowering=False, debug=False, num_devices=H,
        enable_asserts=False,
    )
    _build(nc, C, D, F)
    nc.compile()

    trace_kwargs = (
        {"trace": True, "trace_cores": list(range(H)), "stitch_traces": False}
        if _trace
        else {}
    )
    res = run_bass_kernel_spmd(nc, in_maps, core_ids=list(range(H)), **trace_kwargs)
    if _trace:
        LAST_PROFILE.clear()
        LAST_PROFILE.update(
            exec_time_ns=res.exec_time_ns,
            mean_exec_time_ns=res.mean_exec_time_ns,
            max_exec_time_core_id=res.max_exec_time_core_id,
            trace=res.instructions_and_trace[1] if res.instructions_and_trace else None,
            profile_json=res.profile_json,
        )

    out = np.empty((B, F), dtype=np.float32)
    for h in range(H):
        r = kept[h]
        out[r] = res.results[h]["y"][:len(r)].astype(np.float32)
        if len(over[h]):
            out[over[h]] = x[over[h]] @ wk[h] + bv[h]
    return out


# revision 25
# speedup vs baseline: 1.0026x; 1.0026x over previous
"""MoE routed dense layer (nn_MultiHeadDense): y[b] = x[b] @ W[idx[b]] + bias[idx[b]].

Full shapes: inputs [4096,1024] f32, indices [4096] int, kernel [8,1024,1024] f32,
bias [8,1024] f32 -> out [4096,1024] f32.

Sharding strategy (expert-parallel, H == n_cores == 8): core h owns expert h's
weight [1024,1024] and processes up to C=512 of the rows routed to expert h.
The host computes the per-expert row lists from `indices`, gathers each
expert's first C rows into a zero-padded transposed activation block
XT_h [D, C], and scatters the per-core outputs back into the full [B, F]
result. Rows beyond C on an overloaded expert (~1% of rows for balanced
routing) are computed on the host in f32; this keeps the device at exactly
4 full 128-row m-tiles (64 matmuls) instead of 5 mostly-empty ones.

On-device per core: Y[c, f] = sum_k XT[k*128:(k+1)*128, c].T @ W[k*128:.., f]
accumulated in PSUM over the 8 k-tiles, bias added during the PSUM->SBUF
eviction (cast to fp16 for the output DMA; host upcasts). X and W are
pre-cast to fp16 on the host (11-bit mantissa keeps the absmax error ~1e-3
of output scale while halving HBM traffic); accumulation stays fp32 in PSUM.

Schedule (from trace analysis of the previous versions):
- Input stream = 8 uniform sync-queue DMAs, one [P, F+C] chunk per k-tile
  (W[k] | XT[k]) in consumption order, followed by the bias (needed ~2us
  after the last chunk, so it can't delay the stream head).  Uniform small
  chunks deliver the k6/k7 data ~2us earlier than coarser groupings,
  removing the mid-stream PE stall; per-chunk completion sems pipeline the
  ~1-2us DMA completion-receipt latency.
- Zero-matmul warmup bridges PE activity from queue start until chunk0
  lands so the HAM full-duty clock (2.4 GHz after ~3.4us of sustained PE
  activity) is reached with as few cold real matmuls as possible.
- Phase 1 runs k0..k3 k-outermost (k0 n-outer so only chunk0a gates the
  first matmuls, racing the DMA fill); phase 2 runs k4..k7 m-outermost,
  n-outer within each m, with per-(m, n) evictions into single-bank PSUM
  tiles: DVE adds bias and casts PSUM fp32 -> fp16, then the [128,512]
  piece is DMA'd out immediately, n=0 on the scalar ring and n=1 on the
  sync ring.  Finishing each m's n=0 bank first lets its DVE eviction
  overlap the whole n=1 sub-window, so only the final ~0.73us eviction
  trails the last matmul; small fp16 output pieces shorten the
  last-matmul -> last-DMA-receipt tail that gates the fixed ~8.3us
  framework epilogue.
"""

from contextlib import ExitStack

import numpy as np

import concourse.bass as bass
import concourse.tile as tile
from concourse import bacc, mybir
from concourse.bass_utils import run_bass_kernel_spmd

F32 = mybir.dt.float32
F16 = mybir.dt.float16

P = 128          # SBUF partitions / matmul tile edge
NTILE = 512      # matmul moving free dim (one fp32 PSUM bank)
CAP = 512        # device rows per core; overflow rows computed on host
WARMUP_MM = 7    # zero-matmuls bridging PE idle until chunk0 lands
PHASE1_K = 4     # k0..PHASE1_K-1 run k-outer; the rest run m-outer + evict


def _build(nc: bass.Bass, C: int, D: int, F: int, warmup=WARMUP_MM):
    KT = D // P
    NT = F // NTILE
    MT = C // P
    assert C % P == 0 and NT == 2
    Q = F + C        # columns per full k-tile chunk

    wx = nc.dram_tensor("wx", (KT * P * Q,), F16, kind="ExternalInput").ap()
    bias_d = nc.dram_tensor("bias", (P * F,), F16, kind="ExternalInput").ap()
    y = nc.dram_tensor("y", (C, F), F16, kind="ExternalOutput").ap()

    with tile.TileContext(nc) as tc, ExitStack() as ctx:
        cp = ctx.enter_context(tc.tile_pool(name="cp", bufs=1))
        zp = ctx.enter_context(tc.tile_pool(name="zp", bufs=1))
        pp = ctx.enter_context(tc.tile_pool(name="pp", bufs=1, space="PSUM"))
        yp = ctx.enter_context(tc.tile_pool(name="yp", bufs=1))

        # Input stream on the sync HWDGE ring, in consumption order:
        # chunk0a [P, 512+C] = W[k0][:, :512] | XT[k0]  (k0's n=0 matmuls
        # run for all four m-tiles before anything needs chunk0b, riding
        # out the per-chunk completion-sem lag), chunk0b [P, 512] =
        # W[k0][:, 512:], then one [P, F+C] chunk (W[k] | XT[k]) per k-tile
        # k1..k7.  The bias rides the same ring AFTER the stream (it is
        # first needed ~2us after the last chunk lands) so it cannot delay
        # the head of the stream, and the scalar ring stays free for the
        # n=0 output DMAs.
        sizes = [NTILE + C, NTILE] + [Q] * (KT - 1)
        wx_c = []
        off = 0
        for c, q in enumerate(sizes):
            ct = cp.tile([P, q], F16, name=f"wx{c}", tag=f"wx{c}")
            nc.sync.dma_start(
                ct[:], wx[off:off + P * q].rearrange("(p q) -> p q", p=P))
            wx_c.append(ct)
            off += P * q
        bias_t = cp.tile([P, F], F16, name="bias", tag="bias")
        nc.sync.dma_start(
            bias_t[:], bias_d[:].rearrange("(p q) -> p q", p=P))

        # One single-bank PSUM tile per (m, n) output block: keeps the DVE
        # eviction of bank (m, n=0) free of any (false) dependency against
        # the still-running matmuls into bank (m, n=1).
        ps = [[pp.tile([P, NTILE], F32, name=f"ps{m}_{n}", tag=f"ps{m}_{n}")
               for n in range(NT)] for m in range(MT)]

        # PE warmup: zero matmuls (no DMA dependency) keep the PE busy
        # until chunk0a's completion receipt lands, so the HAM clock-gate
        # warmup overlaps the DMA fill instead of following it. They
        # target ps[0][0], which the first real k=0 matmul resets (start=True).
        zt = zp.tile([P, NTILE], F16)
        nc.vector.memset(zt[:], 0.0)
        for _ in range(warmup):
            nc.tensor.matmul(ps[0][0][:], lhsT=zt[:, :P], rhs=zt[:],
                             start=True, stop=True)

        def mm(m, k, n):
            if k == 0:
                t = wx_c[0] if n == 0 else wx_c[1]
                xt = wx_c[0]
                xbase, wbase = NTILE, 0
            else:
                t = xt = wx_c[k + 1]
                xbase, wbase = F, n * NTILE
            nc.tensor.matmul(
                ps[m][n][:],
                lhsT=xt[:, xbase + m * P:xbase + (m + 1) * P],
                rhs=t[:, wbase:wbase + NTILE],
                start=(k == 0),
                stop=(k == KT - 1),
            )

        def evict(m, n):
            yt = yp.tile([P, NTILE], F16, name=f"yt{m}_{n}", tag=f"yt{m}_{n}")
            nc.vector.tensor_add(
                yt[:],
                ps[m][n][:],
                bias_t[:, n * NTILE:(n + 1) * NTILE],
            )
            eng = nc.scalar if n == 0 else nc.sync
            eng.dma_start(
                y[m * P:(m + 1) * P, n * NTILE:(n + 1) * NTILE], yt[:])

        # k0 runs n-outer so only chunk0a gates the first four matmuls;
        # later k-tiles run n-inner (both n in one chunk).
        for n in range(NT):
            for m in range(MT):
                mm(m, 0, n)
        for k in range(1, PHASE1_K):
            for m in range(MT):
                for n in range(NT):
                    mm(m, k, n)
        # Phase 2, m-outer.  m=0 runs n-inner (k-interleaved) so its first
        # k7 matmul lands ~0.65us later than n-outer would place it --
        # chunk7's completion sem arrives only ~1us before phase 2 starts,
        # and the slack absorbs per-core DMA jitter.  Later m-tiles run
        # n-outer: finishing the n=0 bank first lets its DVE eviction
        # overlap the whole n=1 sub-window, so only the final ~0.7us
        # eviction trails the last matmul.
        for k in range(PHASE1_K, KT):
            for n in range(NT):
                mm(0, k, n)
        evict(0, 0)
        evict(0, 1)
        for m in range(1, MT):
            for n in range(NT):
                for k in range(PHASE1_K, KT):
                    mm(m, k, n)
                evict(m, n)


LAST_PROFILE = {}


def kernel(inputs, indices, kernel, bias, _trace=False):
    x = np.ascontiguousarray(np.asarray(inputs), dtype=np.float32)
    idx = np.asarray(indices).astype(np.int64)
    wk = np.asarray(kernel, dtype=np.float32)
    bv = np.asarray(bias, dtype=np.float32)

    B, D = x.shape
    H, _, F = wk.shape
    C = CAP
    KT = D // P

    rows = [np.nonzero(idx == h)[0] for h in range(H)]
    kept = [r[:C] for r in rows]
    over = [r[C:] for r in rows]

    def pack(w16, xt16):
        # stream layout: chunk0a [P, 512+C] = W[k0][:, :512] | XT[k0]
        #                chunk0b [P, 512]   = W[k0][:, 512:]
        #                chunk k [P, F+C]   = W[k] | XT[k]     (k=1..KT-1)
        w = w16.reshape(KT, P, F)
        xt = xt16.reshape(KT, P, C)
        parts = [
            np.concatenate([w[0, :, :NTILE], xt[0]], axis=1).reshape(-1),
            w[0, :, NTILE:].reshape(-1),
            np.concatenate([w[1:], xt[1:]], axis=2).reshape(-1),
        ]
        return np.concatenate(parts)

    in_maps = []
    for h in range(H):
        r = kept[h]
        xt = np.zeros((D, C), dtype=np.float16)
        xt[:, :len(r)] = x[r].T
        in_maps.append({
            "wx": pack(wk[h].astype(np.float16), xt),
            "bias": np.broadcast_to(bv[h].astype(np.float16), (P, F)).reshape(-1),
        })

    nc = bacc.Bacc(
        "TRN2", target_bir_lowering=False, debug=False, num_devices=H,
        enable_asserts=False,
    )
    _build(nc, C, D, F)
    nc.compile()

    trace_kwargs = (
        {"trace": True, "trace_cores": list(range(H)), "stitch_traces": False}
        if _trace
        else {}
    )
    res = run_bass_kernel_spmd(nc, in_maps, core_ids=list(range(H)), **trace_kwargs)
    if _trace:
        LAST_PROFILE.clear()
        LAST_PROFILE.update(
            exec_time_ns=res.exec_time_ns,
            mean_exec_time_ns=res.mean_exec_time_ns,
            max_exec_time_core_id=res.max_exec_time_core_id,
            trace=res.instructions_and_trace[1] if res.instructions_and_trace else None,
            profile_json=res.profile_json,
        )

    out = np.empty((B, F), dtype=np.float32)
    for h in range(H):
        r = kept[h]
        out[r] = res.results[h]["y"][:len(r)].astype(np.float32)
        if len(over[h]):
            out[over[h]] = x[over[h]] @ wk[h] + bv[h]
    return out


# revision 27
# speedup vs baseline: 1.0083x; 1.0057x over previous
"""MoE routed dense layer (nn_MultiHeadDense): y[b] = x[b] @ W[idx[b]] + bias[idx[b]].

Full shapes: inputs [4096,1024] f32, indices [4096] int, kernel [8,1024,1024] f32,
bias [8,1024] f32 -> out [4096,1024] f32.

Sharding strategy (expert-parallel, H == n_cores == 8): core h owns expert h's
weight [1024,1024] and processes up to C=512 of the rows routed to expert h.
The host computes the per-expert row lists from `indices`, gathers each
expert's first C rows into a zero-padded transposed activation block
XT_h [D, C], and scatters the per-core outputs back into the full [B, F]
result. Rows beyond C on an overloaded expert (~1% of rows for balanced
routing) are computed on the host in f32; this keeps the device at exactly
4 full 128-row m-tiles (64 matmuls) instead of 5 mostly-empty ones.

On-device per core: Y[c, f] = sum_k XT[k*128:(k+1)*128, c].T @ W[k*128:.., f]
accumulated in PSUM over the 8 k-tiles, bias added during the PSUM->SBUF
eviction (cast to fp16 for the output DMA; host upcasts). X and W are
pre-cast to fp16 on the host (11-bit mantissa keeps the absmax error ~1e-3
of output scale while halving HBM traffic); accumulation stays fp32 in PSUM.

Schedule (from trace analysis of the previous versions):
- Input stream = 8 uniform sync-queue DMAs, one [P, F+C] chunk per k-tile
  (W[k] | XT[k]) in consumption order, followed by the bias (needed ~2us
  after the last chunk, so it can't delay the stream head).  Uniform small
  chunks deliver the k6/k7 data ~2us earlier than coarser groupings,
  removing the mid-stream PE stall; per-chunk completion sems pipeline the
  ~1-2us DMA completion-receipt latency.
- Zero-matmul warmup bridges PE activity from queue start until chunk0
  lands so the HAM full-duty clock (2.4 GHz after ~3.4us of sustained PE
  activity) is reached with as few cold real matmuls as possible.
- Phase 1 runs k0..k3 k-outermost (k0 n-outer so only chunk0a gates the
  first matmuls, racing the DMA fill); phase 2 runs k4..k7 m-outermost,
  n-outer within each m, with per-(m, n) evictions into single-bank PSUM
  tiles: DVE adds bias and casts PSUM fp32 -> fp16, then the [128,512]
  piece is DMA'd out immediately, n=0 on the scalar ring and n=1 on the
  sync ring.  Finishing each m's n=0 bank first lets its DVE eviction
  overlap the whole n=1 sub-window, so only the final ~0.73us eviction
  trails the last matmul; small fp16 output pieces shorten the
  last-matmul -> last-DMA-receipt tail that gates the fixed ~8.3us
  framework epilogue.
"""

from contextlib import ExitStack

import numpy as np

import concourse.bass as bass
import concourse.tile as tile
from concourse import bacc, mybir
from concourse.bass_utils import run_bass_kernel_spmd

F32 = mybir.dt.float32
F16 = mybir.dt.float16

P = 128          # SBUF partitions / matmul tile edge
NTILE = 512      # matmul moving free dim (one fp32 PSUM bank)
CAP = 512        # device rows per core; overflow rows computed on host
WARMUP_MM = 7    # zero-matmuls bridging PE idle until chunk0 lands
PHASE1_K = 5     # k0..PHASE1_K-1 run k-outer; the rest run m-outer + evict


def _build(nc: bass.Bass, C: int, D: int, F: int, warmup=WARMUP_MM):
    KT = D // P
    NT = F // NTILE
    MT = C // P
    assert C % P == 0 and NT == 2
    Q = F + C        # columns per full k-tile chunk

    wx = nc.dram_tensor("wx", (KT * P * Q,), F16, kind="ExternalInput").ap()
    bias_d = nc.dram_tensor("bias", (P * F,), F16, kind="ExternalInput").ap()
    y = nc.dram_tensor("y", (C, F), F16, kind="ExternalOutput").ap()

    with tile.TileContext(nc) as tc, ExitStack() as ctx:
        cp = ctx.enter_context(tc.tile_pool(name="cp", bufs=1))
        zp = ctx.enter_context(tc.tile_pool(name="zp", bufs=1))
        pp = ctx.enter_context(tc.tile_pool(name="pp", bufs=1, space="PSUM"))
        yp = ctx.enter_context(tc.tile_pool(name="yp", bufs=1))

        # Input stream on the sync HWDGE ring, in consumption order:
        # chunk0a [P, 512+C] = W[k0][:, :512] | XT[k0]  (k0's n=0 matmuls
        # run for all four m-tiles before anything needs chunk0b, riding
        # out the per-chunk completion-sem lag), chunk0b [P, 512] =
        # W[k0][:, 512:], then one [P, F+C] chunk (W[k] | XT[k]) per k-tile
        # k1..k7.  The bias rides the same ring AFTER the stream (it is
        # first needed ~2us after the last chunk lands) so it cannot delay
        # the head of the stream, and the scalar ring stays free for the
        # n=0 output DMAs.
        sizes = [NTILE + C, NTILE] + [Q] * (KT - 1)
        wx_c = []
        off = 0
        for c, q in enumerate(sizes):
            ct = cp.tile([P, q], F16, name=f"wx{c}", tag=f"wx{c}")
            nc.sync.dma_start(
                ct[:], wx[off:off + P * q].rearrange("(p q) -> p q", p=P))
            wx_c.append(ct)
            off += P * q
        bias_t = cp.tile([P, F], F16, name="bias", tag="bias")
        nc.sync.dma_start(
            bias_t[:], bias_d[:].rearrange("(p q) -> p q", p=P))

        # One single-bank PSUM tile per (m, n) output block: keeps the DVE
        # eviction of bank (m, n=0) free of any (false) dependency against
        # the still-running matmuls into bank (m, n=1).
        ps = [[pp.tile([P, NTILE], F32, name=f"ps{m}_{n}", tag=f"ps{m}_{n}")
               for n in range(NT)] for m in range(MT)]

        # PE warmup: zero matmuls (no DMA dependency) keep the PE busy
        # until chunk0a's completion receipt lands, so the HAM clock-gate
        # warmup overlaps the DMA fill instead of following it. They
        # target ps[0][0], which the first real k=0 matmul resets (start=True).
        zt = zp.tile([P, NTILE], F16)
        nc.vector.memset(zt[:], 0.0)
        for _ in range(warmup):
            nc.tensor.matmul(ps[0][0][:], lhsT=zt[:, :P], rhs=zt[:],
                             start=True, stop=True)

        def mm(m, k, n):
            if k == 0:
                t = wx_c[0] if n == 0 else wx_c[1]
                xt = wx_c[0]
                xbase, wbase = NTILE, 0
            else:
                t = xt = wx_c[k + 1]
                xbase, wbase = F, n * NTILE
            nc.tensor.matmul(
                ps[m][n][:],
                lhsT=xt[:, xbase + m * P:xbase + (m + 1) * P],
                rhs=t[:, wbase:wbase + NTILE],
                start=(k == 0),
                stop=(k == KT - 1),
            )

        def evict(m, n):
            yt = yp.tile([P, NTILE], F16, name=f"yt{m}_{n}", tag=f"yt{m}_{n}")
            nc.vector.tensor_add(
                yt[:],
                ps[m][n][:],
                bias_t[:, n * NTILE:(n + 1) * NTILE],
            )
            eng = nc.scalar if n == 0 else nc.sync
            eng.dma_start(
                y[m * P:(m + 1) * P, n * NTILE:(n + 1) * NTILE], yt[:])

        # k0 runs n-outer so only chunk0a gates the first four matmuls;
        # later k-tiles run n-inner (both n in one chunk).
        for n in range(NT):
            for m in range(MT):
                mm(m, 0, n)
        for k in range(1, PHASE1_K):
            for m in range(MT):
                for n in range(NT):
                    mm(m, k, n)
        # Phase 2, m-outer, n-inner (k-interleaved) for all but the last
        # m-tile: each m's first k7 matmul lands as late as possible, so the
        # tail chunks' completion sems (which arrive only ~1-2us ahead of
        # phase 2) keep their margin against per-core DMA jitter.  The LAST
        # m-tile runs n-outer -- by then every chunk has long landed -- so
        # its n=0 eviction overlaps the n=1 sub-window and only the final
        # ~0.9us eviction trails the last matmul.
        for m in range(MT - 1):
            for k in range(PHASE1_K, KT - 1):
                for n in range(NT):
                    mm(m, k, n)
            mm(m, KT - 1, 0)
            evict(m, 0)
            mm(m, KT - 1, 1)
            evict(m, 1)
        mlast = MT - 1
        for n in range(NT):
            for k in range(PHASE1_K, KT):
                mm(mlast, k, n)
            evict(mlast, n)


LAST_PROFILE = {}


def kernel(inputs, indices, kernel, bias, _trace=False):
    x = np.ascontiguousarray(np.asarray(inputs), dtype=np.float32)
    idx = np.asarray(indices).astype(np.int64)
    wk = np.asarray(kernel, dtype=np.float32)
    bv = np.asarray(bias, dtype=np.float32)

    B, D = x.shape
    H, _, F = wk.shape
    C = CAP
    KT = D // P

    rows = [np.nonzero(idx == h)[0] for h in range(H)]
    kept = [r[:C] for r in rows]
    over = [r[C:] for r in rows]

    def pack(w16, xt16):
        # stream layout: chunk0a [P, 512+C] = W[k0][:, :512] | XT[k0]
        #                chunk0b [P, 512]   = W[k0][:, 512:]
        #                chunk k [P, F+C]   = W[k] | XT[k]     (k=1..KT-1)
        w = w16.reshape(KT, P, F)
        xt = xt16.reshape(KT, P, C)
        parts = [
            np.concatenate([w[0, :, :NTILE], xt[0]], axis=1).reshape(-1),
            w[0, :, NTILE:].reshape(-1),
            np.concatenate([w[1:], xt[1:]], axis=2).reshape(-1),
        ]
        return np.concatenate(parts)

    in_maps = []
    for h in range(H):
        r = kept[h]
        xt = np.zeros((D, C), dtype=np.float16)
        xt[:, :len(r)] = x[r].T
        in_maps.append({
            "wx": pack(wk[h].astype(np.float16), xt),
            "bias": np.broadcast_to(bv[h].astype(np.float16), (P, F)).reshape(-1),
        })

    nc = bacc.Bacc(
        "TRN2", target_bir_lowering=False, debug=False, num_devices=H,
        enable_asserts=False,
    )
    _build(nc, C, D, F)
    nc.compile()

    trace_kwargs = (
        {"trace": True, "trace_cores": list(range(H)), "stitch_traces": False}
        if _trace
        else {}
    )
    res = run_bass_kernel_spmd(nc, in_maps, core_ids=list(range(H)), **trace_kwargs)
    if _trace:
        LAST_PROFILE.clear()
        LAST_PROFILE.update(
            exec_time_ns=res.exec_time_ns,
            mean_exec_time_ns=res.mean_exec_time_ns,
            max_exec_time_core_id=res.max_exec_time_core_id,
            trace=res.instructions_and_trace[1] if res.instructions_and_trace else None,
            profile_json=res.profile_json,
        )

    out = np.empty((B, F), dtype=np.float32)
    for h in range(H):
        r = kept[h]
        out[r] = res.results[h]["y"][:len(r)].astype(np.float32)
        if len(over[h]):
            out[over[h]] = x[over[h]] @ wk[h] + bv[h]
    return out
